# revision 55
# baseline (speedup 1.0000x reference)
"""ConvGRUBandCell2d fused Trainium2 kernel (8 NeuronCores, batch-parallel).

v3: fp8e4 DoubleRow matmuls for the gate GEMMs and the ssq (rmsnorm)
reductions. PE work drops ~1.44x vs bf16: each DoubleRow matmul consumes
two 128-channel k-tiles at 0.5 cyc/row. Activations/weights are scaled
into fp8 range with static scales; the descale (1/S) folds into the
sigmoid/tanh activation `scale` so no extra elementwise ops appear.

  A  = 16    x-activation quant scale (xs8 = x*invx*A)
  T  = 16    hm quant scale (folded into the depthwise taps)
  W  = 1024  weight quant scale (both xW and hW)
  S  = A*W = T*W = 16384  PSUM descale

Numerics (numpy-simulated, matches HW bf16 run exactly at 6.71e-3):
full-fp8 incl. fp8 squares predicts rel-err 1.46e-2 < 2e-2 gate.

DVE diet: inv tensors are cast to bf16 so the h*inv / y*inv tensor_tensor
ops hit the DVE 2x_1P mode; depthwise taps stored bf16.

Pipeline structure is identical to v2 (3-stage software pipeline over
batches; emission order per iteration i: L(i+1), Y(i-2), F(i), B(i-1)).
"""

import numpy as np

B, C, F, K = 64, 512, 1024, 3
N_CORES = 8
BPC = B // N_CORES          # batches per core
TC = C // 128               # channel tiles (4)
M3 = (3 * C) // 128         # gate-row tiles (12)
EPS = 1e-6
F1_POS = 1
F2_POS = 2
A_SC = 16.0
W_SC = 1024.0
T_SC = 16.0
S_SC = A_SC * W_SC

_CACHE = {}


def _build_program(use_won):
    import concourse.bacc as bacc
    import concourse.tile as tile
    from concourse import mybir

    f32 = mybir.dt.float32
    bf16 = mybir.dt.bfloat16
    fp8 = mybir.dt.float8e4
    AF = mybir.ActivationFunctionType
    OP = mybir.AluOpType
    DR = mybir.MatmulPerfMode.DoubleRow

    nc = bacc.Bacc("TRN2", target_bir_lowering=False, debug=False,
                   num_devices=N_CORES)

    xd = nc.dram_tensor("x", [BPC, C, F], bf16, kind="ExternalInput").ap()
    hd = nc.dram_tensor("h", [BPC, C, F], bf16, kind="ExternalInput").ap()
    # fp8 weights, layout [p, ktile, m] so [:, 2j:2j+2, mslice] is a
    # DoubleRow lhsT
    xW8d = nc.dram_tensor("xW8", [128, TC, 3 * C], fp8,
                          kind="ExternalInput").ap()
    hW8d = nc.dram_tensor("hW8", [128, TC, 3 * C], fp8,
                          kind="ExternalInput").ap()
    w3d = nc.dram_tensor("w3", [C, K], f32, kind="ExternalInput").ap()
    gbd = nc.dram_tensor("gb", [3 * C, 1], f32, kind="ExternalInput").ap()
    bhnd = nc.dram_tensor("bhn", [C, 1], f32, kind="ExternalInput").ap()
    xbnd = nc.dram_tensor("xbn", [C, 1], f32, kind="ExternalInput").ap()
    wond = nc.dram_tensor("won", [C, 1], f32, kind="ExternalInput").ap()
    onesd = nc.dram_tensor("ones_in", [128, 2, 128], fp8,
                           kind="ExternalInput").ap()
    eyed = nc.dram_tensor("eye_in", [128, 128], bf16,
                          kind="ExternalInput").ap()
    outd = nc.dram_tensor("out", [BPC, C, F], bf16, kind="ExternalOutput").ap()

    CHS = [slice(0, 512), slice(512, 1024)]

    with tile.TileContext(nc) as tc:
        with (
            tc.tile_pool(name="wp", bufs=1) as wp,
            tc.tile_pool(name="sb", bufs=2) as sb,
            tc.tile_pool(name="prz", bufs=2, space="PSUM") as prz,
            tc.tile_pool(name="pch", bufs=4, space="PSUM") as pch,
        ):
            # ---- resident weights / constants ----
            xw8 = wp.tile([128, TC, 3 * C], fp8, tag="xw8", name="xw8")
            nc.sync.dma_start(xw8[:], xW8d[:, :, :])
            hw8 = wp.tile([128, TC, 3 * C], fp8, tag="hw8", name="hw8")
            nc.sync.dma_start(hw8[:], hW8d[:, :, :])
            w3t = []
            for k in range(TC):
                w3 = wp.tile([128, K], f32, tag=f"w3{k}", name=f"w3{k}")
                nc.sync.dma_start(w3[:], w3d[k * 128:(k + 1) * 128, :])
                w3t.append(w3)
            ones8 = wp.tile([128, 2, 128], fp8, tag="ones8", name="ones8")
            nc.sync.dma_start(ones8[:], onesd[:, :, :])
            eye = wp.tile([128, 128], bf16, tag="eye", name="eye")
            nc.sync.dma_start(eye[:], eyed[:, :])
            gbt = wp.tile([128, M3], f32, tag="gbt", name="gbt")
            nc.sync.dma_start(gbt[:], gbd.rearrange("(m p) o -> p (m o)", p=128))
            bhnt = wp.tile([128, TC], f32, tag="bhnt", name="bhnt")
            nc.sync.dma_start(bhnt[:], bhnd.rearrange("(m p) o -> p (m o)", p=128))
            xbnt = wp.tile([128, TC], f32, tag="xbnt", name="xbnt")
            nc.sync.dma_start(xbnt[:], xbnd.rearrange("(m p) o -> p (m o)", p=128))
            wont = wp.tile([128, TC], f32, tag="wont", name="wont")
            nc.sync.dma_start(wont[:], wond.rearrange("(m p) o -> p (m o)", p=128))
            epst = wp.tile([128, 1], f32, tag="epst", name="epst")
            nc.vector.memset(epst[:], EPS)
            epsA = wp.tile([128, 1], f32, tag="epsA", name="epsA")
            nc.vector.memset(epsA[:], EPS / (A_SC * A_SC))

            st = [dict() for _ in range(BPC)]

            def stage_L(b):
                s = st[b]
                s["ht"] = []
                s["xt"] = []
                for ct in range(TC):
                    t = sb.tile([128, F], bf16, tag=f"ht{ct}", name=f"ht{b}_{ct}")
                    nc.sync.dma_start(t[:], hd[b, ct * 128:(ct + 1) * 128, :])
                    s["ht"].append(t)
                for ct in range(TC):
                    t = sb.tile([128, F], bf16, tag=f"xt{ct}", name=f"xt{b}_{ct}")
                    nc.sync.dma_start(t[:], xd[b, ct * 128:(ct + 1) * 128, :])
                    s["xt"].append(t)

            def norm_sq(src_tiles, nm, b):
                """fp8 squares (ACT) into a [128,TC,F] tile."""
                sq = sb.tile([128, TC, F], fp8, tag="sq8", bufs=3,
                             name=f"sq{nm}{b}")
                for ct in range(TC):
                    nc.scalar.square(sq[:, ct, :], src_tiles[ct][:])
                return sq

            def norm_mm(sq, nm, b):
                """DoubleRow ssq MMs. Returns two [128,512] PSUM tiles."""
                pts = pch.tile([128, 512], f32, tag="chk", bufs=4,
                               name=f"n{nm}{b}_0")
                pts2 = pch.tile([128, 512], f32, tag="chk", bufs=4,
                                name=f"n{nm}{b}_1")
                pp = [pts, pts2]
                for jp in range(2):
                    for ch in range(2):
                        nc.tensor.matmul(pp[ch][:], ones8[:, :, :],
                                         sq[:, 2 * jp:2 * jp + 2, CHS[ch]],
                                         start=(jp == 0), stop=(jp == 1),
                                         perf_mode=DR)
                return pp

            def norm_sqrt(pp, nm, b, scale, bias_t):
                """sqrt (ACT) over the two PSUM chunks -> sr f32 [128,F].
                Frees the PSUM chunk tiles immediately."""
                sr = sb.tile([128, F], f32, tag="sr", bufs=3, name=f"sr{nm}{b}")
                for ch in range(2):
                    nc.scalar.activation(sr[:, CHS[ch]], pp[ch][:], AF.Sqrt,
                                         bias=bias_t[:], scale=scale)
                return sr

            def norm_recip(sr, nm, b):
                """recip (DVE f32) -> cast bf16. Returns inv bf16 [128,F]."""
                inv = sb.tile([128, F], f32, tag="invf", bufs=3,
                              name=f"inv{nm}{b}")
                nc.vector.reciprocal_approx_fast(inv[:], sr[:])
                invb = sb.tile([128, F], bf16, tag=f"inv{nm}", bufs=1,
                               name=f"invb{nm}{b}")
                nc.vector.tensor_scalar_mul(invb[:], inv[:], 1.0)
                return invb

            def stage_F1(b):
                s = st[b]
                s["srh"] = norm_sqrt(norm_mm(norm_sq(s["ht"], "h", b),
                                             "h", b), "h", b, 1.0 / C, epst)
                s["srx"] = norm_sqrt(norm_mm(norm_sq(s["xt"], "x", b),
                                             "x", b), "x", b,
                                     1.0 / (C * A_SC * A_SC), epsA)

            def stage_Y1(b):
                s = st[b]
                s["sry"] = norm_sqrt(norm_mm(norm_sq(s["y"], "y", b),
                                             "y", b), "y", b, 1.0 / C, epst)

            def stage_F2(b):
                s = st[b]
                invh = norm_recip(s["srh"], "h", b)
                invx = norm_recip(s["srx"], "x", b)
                s["hs"] = []
                for ct in range(TC):
                    t = sb.tile([128, F + 2], bf16, tag=f"hs{ct}", bufs=1,
                                name=f"hs{b}_{ct}")
                    if b == 0:
                        nc.vector.memset(t[:, 0:1], 0.0)
                        nc.vector.memset(t[:, F + 1:F + 2], 0.0)
                    nc.vector.tensor_mul(t[:, 1:F + 1], s["ht"][ct][:], invh[:])
                    s["hs"].append(t)
                xs8 = sb.tile([128, TC, F], fp8, tag="xs8", bufs=2,
                              name=f"xs8{b}")
                for ct in range(TC):
                    nc.vector.tensor_mul(xs8[:, ct, :], s["xt"][ct][:],
                                         invx[:])
                s["xs8"] = xs8
                hm8 = sb.tile([128, TC, F], fp8, tag="hm8", bufs=2,
                              name=f"hm8{b}")
                for ct in range(TC):
                    hs = s["hs"][ct]
                    t = sb.tile([128, F], bf16, tag="hmt", bufs=2,
                                name=f"hmt{b}_{ct}")
                    nc.vector.tensor_scalar_mul(t[:], hs[:, 1:F + 1],
                                                w3t[ct][:, 1:2])
                    nc.vector.scalar_tensor_tensor(
                        t[:], hs[:, 0:F], w3t[ct][:, 0:1], t[:],
                        OP.mult, OP.add)
                    nc.vector.scalar_tensor_tensor(
                        hm8[:, ct, :], hs[:, 2:F + 2], w3t[ct][:, 2:3], t[:],
                        OP.mult, OP.add)
                s["hm8"] = hm8

            def emit_rz_gate(b, m):
                """One [128,1024] PSUM chain: per 512-chunk, 2 DoubleRow
                xW MMs + 2 DoubleRow hW MMs; then one sigmoid (descale
                folded into the activation scale)."""
                s = st[b]
                ps = prz.tile([128, F], f32, tag="rz", name=f"rz{b}_{m}")
                ms = slice(m * 128, (m + 1) * 128)
                for jp in range(2):
                    kp = slice(2 * jp, 2 * jp + 2)
                    for ch in range(2):
                        nc.tensor.matmul(ps[:, CHS[ch]], xw8[:, kp, ms],
                                         s["xs8"][:, kp, CHS[ch]],
                                         start=(jp == 0), stop=False,
                                         perf_mode=DR)
                for jp in range(2):
                    kp = slice(2 * jp, 2 * jp + 2)
                    for ch in range(2):
                        nc.tensor.matmul(ps[:, CHS[ch]], hw8[:, kp, ms],
                                         s["hm8"][:, kp, CHS[ch]],
                                         start=False, stop=(jp == 1),
                                         perf_mode=DR)
                if m < 4:
                    g = sb.tile([128, F], bf16, tag=f"rg{m}", bufs=1,
                                name=f"rg{b}_{m}")
                    s["rg"].append(g)
                else:
                    g = s["ug"][m - 4]
                nc.scalar.activation(g[:], ps[:], AF.Sigmoid,
                                     bias=gbt[:, m:m + 1], scale=1.0 / S_SC)

            def emit_nA(b, j, ch):
                """psx/psh DoubleRow chains + the r-mult STT. psx is left
                open; emit_nB finishes it (eye-MM accum) later so the PE
                queue never head-of-line blocks on the DVE STT."""
                s = st[b]
                S = CHS[ch]
                ms = slice((8 + j) * 128, (9 + j) * 128)
                psx = pch.tile([128, 512], f32, tag="chk",
                               name=f"npx{b}_{j}_{ch}")
                for jp in range(2):
                    kp = slice(2 * jp, 2 * jp + 2)
                    nc.tensor.matmul(psx[:], xw8[:, kp, ms],
                                     s["xs8"][:, kp, S],
                                     start=(jp == 0), stop=False,
                                     perf_mode=DR)
                psh = pch.tile([128, 512], f32, tag="chk",
                               name=f"nph{b}_{j}_{ch}")
                for jp in range(2):
                    kp = slice(2 * jp, 2 * jp + 2)
                    nc.tensor.matmul(psh[:], hw8[:, kp, ms],
                                     s["hm8"][:, kp, S],
                                     start=(jp == 0), stop=(jp == 1),
                                     perf_mode=DR)
                t = sb.tile([128, 512], bf16, tag="nt", bufs=3,
                            name=f"nt{b}_{j}_{ch}")
                # t = (psh + S*bhn) * r ; psx += I@t (PE) ;
                # cg = tanh(psx/S + xbn)
                nc.vector.scalar_tensor_tensor(
                    t[:], psh[:], bhnt[:, j:j + 1], s["rg"][j][:, S],
                    OP.add, OP.mult)
                s[f"nt{j}_{ch}"] = (psx, t)

            def emit_nB(b, j, ch):
                s = st[b]
                psx, t = s.pop(f"nt{j}_{ch}")
                nc.tensor.matmul(psx[:], eye[:, :], t[:],
                                 start=False, stop=True)
                nc.scalar.activation(s["cg"][j][:, CHS[ch]], psx[:], AF.Tanh,
                                     bias=xbnt[:, j:j + 1], scale=1.0 / S_SC)

            def stage_B(b, f1=None, f2=None, y1=None, y2=None):
                s = st[b]
                s["rg"] = []
                s["ug"] = [sb.tile([128, F], bf16, tag=f"ug{j}", bufs=1,
                                   name=f"ug{b}_{j}") for j in range(4)]
                s["cg"] = [sb.tile([128, F], bf16, tag=f"cg{j}", bufs=1,
                                   name=f"cg{b}_{j}") for j in range(4)]
                for m in range(F1_POS):
                    emit_rz_gate(b, m)
                if f1 is not None:
                    f1()
                for m in range(F1_POS, 5):
                    emit_rz_gate(b, m)
                emit_nA(b, 0, 0)
                emit_nA(b, 0, 1)
                emit_rz_gate(b, 5)
                emit_nB(b, 0, 0)
                emit_nB(b, 0, 1)
                if f2 is not None and F2_POS == 0:
                    f2()
                emit_nA(b, 1, 0)
                emit_nA(b, 1, 1)
                emit_rz_gate(b, 6)
                emit_nB(b, 1, 0)
                emit_nB(b, 1, 1)
                if f2 is not None and F2_POS == 1:
                    f2()
                emit_nA(b, 2, 0)
                emit_nA(b, 2, 1)
                emit_rz_gate(b, 7)
                emit_nB(b, 2, 0)
                emit_nB(b, 2, 1)
                if f2 is not None and F2_POS == 2:
                    f2()
                emit_nA(b, 3, 0)
                emit_nA(b, 3, 1)
                emit_nB(b, 3, 0)
                emit_nB(b, 3, 1)
                # h_new + x_t  (sub/mul/add on GpSimd, last add on DVE)
                s["y"] = []
                for ct in range(TC):
                    y = sb.tile([128, F], bf16, tag=f"yt{ct}", name=f"yt{b}_{ct}")
                    nc.gpsimd.tensor_sub(y[:], s["ht"][ct][:], s["cg"][ct][:])
                    nc.gpsimd.tensor_mul(y[:], y[:], s["ug"][ct][:])
                    nc.vector.tensor_add(y[:], y[:], s["cg"][ct][:])
                    nc.vector.tensor_add(y[:], y[:], s["xt"][ct][:])
                    s["y"].append(y)

            def stage_Y2(b):
                s = st[b]
                invyb = norm_recip(s["sry"], "y", b)
                for ct in range(TC):
                    if use_won:
                        src = sb.tile([128, F], bf16, tag="yw", bufs=2,
                                      name=f"yw{b}_{ct}")
                        nc.vector.tensor_scalar_mul(src[:], s["y"][ct][:],
                                                    wont[:, ct:ct + 1])
                    else:
                        src = s["y"][ct]
                    o = sb.tile([128, F], bf16, tag="ot", bufs=3,
                                name=f"ot{b}_{ct}")
                    nc.vector.tensor_mul(o[:], src[:], invyb[:])
                    nc.sync.dma_start(
                        outd[b, ct * 128:(ct + 1) * 128, :], o[:])
                st[b] = {}

            # Emission order: the B-stage leads; the F-chain for the next
            # batch is woven into it (F1 after rz2 so its ssq MMs + sqrt
            # run mid-B; F2 after the second n-round) so the V queue
            # reaches F2's ops before the iteration tail.
            stage_L(0)
            for i in range(BPC + 2):
                if i + 1 < BPC:
                    stage_L(i + 1)
                f1 = (lambda i=i: stage_F1(i)) if i < BPC else None
                f2 = (lambda i=i: stage_F2(i)) if i < BPC else None
                if 1 <= i <= BPC:
                    stage_B(i - 1, f1=f1, f2=f2)
                else:
                    for fn in (f1, f2):
                        if fn is not None:
                            fn()
                if i >= 2:
                    stage_Y1(i - 2)
                    stage_Y2(i - 2)

    nc.compile()
    return nc


def _get_program(use_won):
    key = ("nc", use_won)
    if key not in _CACHE:
        _CACHE[key] = _build_program(use_won)
    return _CACHE[key]


def kernel(x_t, h_prev, in_norm_w, hid_norm_w, out_norm_w,
           xW, xb, hmixW, hmixb, hW, hb):
    import ml_dtypes
    from concourse.bass_utils import run_bass_kernel_spmd

    use_won = not np.allclose(np.asarray(out_norm_w, np.float32), 1.0)
    nc = _get_program(use_won)

    f = np.float32
    b16 = ml_dtypes.bfloat16
    f8 = ml_dtypes.float8_e4m3
    x = np.ascontiguousarray(np.asarray(x_t, f).reshape(B, C, F).astype(b16))
    h = np.ascontiguousarray(np.asarray(h_prev, f).reshape(B, C, F).astype(b16))
    xW = np.asarray(xW, f)
    hW = np.asarray(hW, f)

    def quant_w(wT):
        # [C, 3C] scaled, fp8, laid out [128, TC, 3C] with p fastest
        q = np.clip(wT * W_SC, -224.0, 224.0).astype(f8)
        return np.ascontiguousarray(
            q.reshape(TC, 128, 3 * C).transpose(1, 0, 2))

    xWT8 = quant_w((xW * np.asarray(in_norm_w, f)[None, :]).T)
    hWT8 = quant_w(hW.T)
    w3 = np.ascontiguousarray(
        (np.asarray(hmixW, f)[:, 0, 0, :]
         * np.asarray(hid_norm_w, f)[:, None]) * T_SC)
    bh = hW @ np.asarray(hmixb, f) + np.asarray(hb, f)
    gb = np.ascontiguousarray((np.asarray(xb, f) + bh).reshape(3 * C, 1))
    bhn = np.ascontiguousarray(S_SC * bh[2 * C:].reshape(C, 1))
    xbn = np.ascontiguousarray(np.asarray(xb, f)[2 * C:].reshape(C, 1))
    won = np.ascontiguousarray(np.asarray(out_norm_w, f).reshape(C, 1))

    shared = {"xW8": xWT8, "hW8": hWT8, "w3": w3, "gb": gb, "bhn": bhn,
              "xbn": xbn, "won": won,
              "ones_in": np.ones((128, 2, 128), dtype=f8),
              "eye_in": np.eye(128, dtype=b16)}
    in_maps = []
    for c in range(N_CORES):
        m = dict(shared)
        m["x"] = x[c * BPC:(c + 1) * BPC]
        m["h"] = h[c * BPC:(c + 1) * BPC]
        in_maps.append(m)

    res = run_bass_kernel_spmd(nc, in_maps, core_ids=list(range(N_CORES)),
                               **_CACHE.get("run_kwargs", {}))
    _CACHE["last_results"] = res
    out = np.concatenate([res.results[c]["out"] for c in range(N_CORES)],
                         axis=0)
    return out.reshape(B, C, 1, F).astype(np.float32)


# revision 56
# speedup vs baseline: 1.0115x; 1.0115x over previous
"""ConvGRUBandCell2d fused Trainium2 kernel (8 NeuronCores, batch-parallel).

v3: fp8e4 DoubleRow matmuls for the gate GEMMs and the ssq (rmsnorm)
reductions. PE work drops ~1.44x vs bf16: each DoubleRow matmul consumes
two 128-channel k-tiles at 0.5 cyc/row. Activations/weights are scaled
into fp8 range with static scales; the descale (1/S) folds into the
sigmoid/tanh activation `scale` so no extra elementwise ops appear.

  A  = 16    x-activation quant scale (xs8 = x*invx*A)
  T  = 16    hm quant scale (folded into the depthwise taps)
  W  = 1024  weight quant scale (both xW and hW)
  S  = A*W = T*W = 16384  PSUM descale

Numerics (numpy-simulated, matches HW bf16 run exactly at 6.71e-3):
full-fp8 incl. fp8 squares predicts rel-err 1.46e-2 < 2e-2 gate.

DVE diet: inv tensors are cast to bf16 so the h*inv / y*inv tensor_tensor
ops hit the DVE 2x_1P mode; depthwise taps stored bf16.

Pipeline structure is identical to v2 (3-stage software pipeline over
batches; emission order per iteration i: L(i+1), Y(i-2), F(i), B(i-1)).
"""

import numpy as np

B, C, F, K = 64, 512, 1024, 3
N_CORES = 8
BPC = B // N_CORES          # batches per core
TC = C // 128               # channel tiles (4)
M3 = (3 * C) // 128         # gate-row tiles (12)
EPS = 1e-6
F1_POS = 1
F2_POS = 1
A_SC = 16.0
W_SC = 1024.0
T_SC = 16.0
S_SC = A_SC * W_SC

_CACHE = {}


def _build_program(use_won):
    import concourse.bacc as bacc
    import concourse.tile as tile
    from concourse import mybir

    f32 = mybir.dt.float32
    bf16 = mybir.dt.bfloat16
    fp8 = mybir.dt.float8e4
    AF = mybir.ActivationFunctionType
    OP = mybir.AluOpType
    DR = mybir.MatmulPerfMode.DoubleRow

    nc = bacc.Bacc("TRN2", target_bir_lowering=False, debug=False,
                   num_devices=N_CORES)

    xd = nc.dram_tensor("x", [BPC, C, F], bf16, kind="ExternalInput").ap()
    hd = nc.dram_tensor("h", [BPC, C, F], bf16, kind="ExternalInput").ap()
    # fp8 weights, layout [p, ktile, m] so [:, 2j:2j+2, mslice] is a
    # DoubleRow lhsT
    xW8d = nc.dram_tensor("xW8", [128, TC, 3 * C], fp8,
                          kind="ExternalInput").ap()
    hW8d = nc.dram_tensor("hW8", [128, TC, 3 * C], fp8,
                          kind="ExternalInput").ap()
    w3d = nc.dram_tensor("w3", [C, K], f32, kind="ExternalInput").ap()
    gbd = nc.dram_tensor("gb", [3 * C, 1], f32, kind="ExternalInput").ap()
    bhnd = nc.dram_tensor("bhn", [C, 1], f32, kind="ExternalInput").ap()
    xbnd = nc.dram_tensor("xbn", [C, 1], f32, kind="ExternalInput").ap()
    wond = nc.dram_tensor("won", [C, 1], f32, kind="ExternalInput").ap()
    onesd = nc.dram_tensor("ones_in", [128, 2, 128], fp8,
                           kind="ExternalInput").ap()
    eyed = nc.dram_tensor("eye_in", [128, 128], bf16,
                          kind="ExternalInput").ap()
    outd = nc.dram_tensor("out", [BPC, C, F], bf16, kind="ExternalOutput").ap()

    CHS = [slice(0, 512), slice(512, 1024)]

    with tile.TileContext(nc) as tc:
        with (
            tc.tile_pool(name="wp", bufs=1) as wp,
            tc.tile_pool(name="sb", bufs=2) as sb,
            tc.tile_pool(name="prz", bufs=2, space="PSUM") as prz,
            tc.tile_pool(name="pch", bufs=4, space="PSUM") as pch,
        ):
            # ---- resident weights / constants ----
            xw8 = wp.tile([128, TC, 3 * C], fp8, tag="xw8", name="xw8")
            nc.sync.dma_start(xw8[:], xW8d[:, :, :])
            hw8 = wp.tile([128, TC, 3 * C], fp8, tag="hw8", name="hw8")
            nc.sync.dma_start(hw8[:], hW8d[:, :, :])
            w3t = []
            for k in range(TC):
                w3 = wp.tile([128, K], f32, tag=f"w3{k}", name=f"w3{k}")
                nc.sync.dma_start(w3[:], w3d[k * 128:(k + 1) * 128, :])
                w3t.append(w3)
            ones8 = wp.tile([128, 2, 128], fp8, tag="ones8", name="ones8")
            nc.sync.dma_start(ones8[:], onesd[:, :, :])
            eye = wp.tile([128, 128], bf16, tag="eye", name="eye")
            nc.sync.dma_start(eye[:], eyed[:, :])
            gbt = wp.tile([128, M3], f32, tag="gbt", name="gbt")
            nc.sync.dma_start(gbt[:], gbd.rearrange("(m p) o -> p (m o)", p=128))
            bhnt = wp.tile([128, TC], f32, tag="bhnt", name="bhnt")
            nc.sync.dma_start(bhnt[:], bhnd.rearrange("(m p) o -> p (m o)", p=128))
            xbnt = wp.tile([128, TC], f32, tag="xbnt", name="xbnt")
            nc.sync.dma_start(xbnt[:], xbnd.rearrange("(m p) o -> p (m o)", p=128))
            wont = wp.tile([128, TC], f32, tag="wont", name="wont")
            nc.sync.dma_start(wont[:], wond.rearrange("(m p) o -> p (m o)", p=128))
            epst = wp.tile([128, 1], f32, tag="epst", name="epst")
            nc.vector.memset(epst[:], EPS)
            epsA = wp.tile([128, 1], f32, tag="epsA", name="epsA")
            nc.vector.memset(epsA[:], EPS / (A_SC * A_SC))

            st = [dict() for _ in range(BPC)]

            def stage_L(b):
                s = st[b]
                s["ht"] = []
                s["xt"] = []
                for ct in range(TC):
                    t = sb.tile([128, F], bf16, tag=f"ht{ct}", name=f"ht{b}_{ct}")
                    nc.sync.dma_start(t[:], hd[b, ct * 128:(ct + 1) * 128, :])
                    s["ht"].append(t)
                for ct in range(TC):
                    t = sb.tile([128, F], bf16, tag=f"xt{ct}", name=f"xt{b}_{ct}")
                    nc.sync.dma_start(t[:], xd[b, ct * 128:(ct + 1) * 128, :])
                    s["xt"].append(t)

            def norm_sq(src_tiles, nm, b):
                """fp8 squares (ACT) into a [128,TC,F] tile."""
                sq = sb.tile([128, TC, F], fp8, tag="sq8", bufs=4,
                             name=f"sq{nm}{b}")
                for ct in range(TC):
                    nc.scalar.square(sq[:, ct, :], src_tiles[ct][:])
                return sq

            def norm_mm(sq, nm, b):
                """DoubleRow ssq MMs. Returns two [128,512] PSUM tiles."""
                pts = pch.tile([128, 512], f32, tag="chk", bufs=4,
                               name=f"n{nm}{b}_0")
                pts2 = pch.tile([128, 512], f32, tag="chk", bufs=4,
                                name=f"n{nm}{b}_1")
                pp = [pts, pts2]
                for jp in range(2):
                    for ch in range(2):
                        nc.tensor.matmul(pp[ch][:], ones8[:, :, :],
                                         sq[:, 2 * jp:2 * jp + 2, CHS[ch]],
                                         start=(jp == 0), stop=(jp == 1),
                                         perf_mode=DR)
                return pp

            def norm_sqrt(pp, nm, b, scale, bias_t):
                """sqrt (ACT) over the two PSUM chunks -> sr f32 [128,F].
                Frees the PSUM chunk tiles immediately."""
                sr = sb.tile([128, F], f32, tag="sr", bufs=3, name=f"sr{nm}{b}")
                for ch in range(2):
                    nc.scalar.activation(sr[:, CHS[ch]], pp[ch][:], AF.Sqrt,
                                         bias=bias_t[:], scale=scale)
                return sr

            def norm_recip(sr, nm, b):
                """recip (DVE f32) -> cast bf16. Returns inv bf16 [128,F]."""
                inv = sb.tile([128, F], f32, tag="invf", bufs=3,
                              name=f"inv{nm}{b}")
                nc.vector.reciprocal_approx_fast(inv[:], sr[:])
                invb = sb.tile([128, F], bf16, tag=f"inv{nm}", bufs=1,
                               name=f"invb{nm}{b}")
                nc.vector.tensor_scalar_mul(invb[:], inv[:], 1.0)
                return invb

            def stage_F1(b):
                s = st[b]
                s["srh"] = norm_sqrt(norm_mm(norm_sq(s["ht"], "h", b),
                                             "h", b), "h", b, 1.0 / C, epst)
                s["srx"] = norm_sqrt(norm_mm(norm_sq(s["xt"], "x", b),
                                             "x", b), "x", b,
                                     1.0 / (C * A_SC * A_SC), epsA)

            def stage_Y1(b):
                s = st[b]
                s["sry"] = norm_sqrt(norm_mm(norm_sq(s["y"], "y", b),
                                             "y", b), "y", b, 1.0 / C, epst)

            def stage_F2(b):
                s = st[b]
                invh = norm_recip(s["srh"], "h", b)
                invx = norm_recip(s["srx"], "x", b)
                s["hs"] = []
                for ct in range(TC):
                    t = sb.tile([128, F + 2], bf16, tag=f"hs{ct}", bufs=1,
                                name=f"hs{b}_{ct}")
                    if b == 0:
                        nc.vector.memset(t[:, 0:1], 0.0)
                        nc.vector.memset(t[:, F + 1:F + 2], 0.0)
                    nc.vector.tensor_mul(t[:, 1:F + 1], s["ht"][ct][:], invh[:])
                    s["hs"].append(t)
                xs8 = sb.tile([128, TC, F], fp8, tag="xs8", bufs=2,
                              name=f"xs8{b}")
                for ct in range(TC):
                    nc.vector.tensor_mul(xs8[:, ct, :], s["xt"][ct][:],
                                         invx[:])
                s["xs8"] = xs8
                hm8 = sb.tile([128, TC, F], fp8, tag="hm8", bufs=2,
                              name=f"hm8{b}")
                for ct in range(TC):
                    hs = s["hs"][ct]
                    t = sb.tile([128, F], bf16, tag="hmt", bufs=2,
                                name=f"hmt{b}_{ct}")
                    nc.vector.tensor_scalar_mul(t[:], hs[:, 1:F + 1],
                                                w3t[ct][:, 1:2])
                    nc.vector.scalar_tensor_tensor(
                        t[:], hs[:, 0:F], w3t[ct][:, 0:1], t[:],
                        OP.mult, OP.add)
                    nc.vector.scalar_tensor_tensor(
                        hm8[:, ct, :], hs[:, 2:F + 2], w3t[ct][:, 2:3], t[:],
                        OP.mult, OP.add)
                s["hm8"] = hm8

            def emit_rz_gate(b, m):
                """One [128,1024] PSUM chain: per 512-chunk, 2 DoubleRow
                xW MMs + 2 DoubleRow hW MMs; then one sigmoid (descale
                folded into the activation scale)."""
                s = st[b]
                ps = prz.tile([128, F], f32, tag="rz", name=f"rz{b}_{m}")
                ms = slice(m * 128, (m + 1) * 128)
                for jp in range(2):
                    kp = slice(2 * jp, 2 * jp + 2)
                    for ch in range(2):
                        nc.tensor.matmul(ps[:, CHS[ch]], xw8[:, kp, ms],
                                         s["xs8"][:, kp, CHS[ch]],
                                         start=(jp == 0), stop=False,
                                         perf_mode=DR)
                for jp in range(2):
                    kp = slice(2 * jp, 2 * jp + 2)
                    for ch in range(2):
                        nc.tensor.matmul(ps[:, CHS[ch]], hw8[:, kp, ms],
                                         s["hm8"][:, kp, CHS[ch]],
                                         start=False, stop=(jp == 1),
                                         perf_mode=DR)
                if m < 4:
                    g = sb.tile([128, F], bf16, tag=f"rg{m}", bufs=1,
                                name=f"rg{b}_{m}")
                    s["rg"].append(g)
                else:
                    g = s["ug"][m - 4]
                nc.scalar.activation(g[:], ps[:], AF.Sigmoid,
                                     bias=gbt[:, m:m + 1], scale=1.0 / S_SC)

            def emit_nA(b, j, ch):
                """psx/psh DoubleRow chains + the r-mult STT. psx is left
                open; emit_nB finishes it (eye-MM accum) later so the PE
                queue never head-of-line blocks on the DVE STT."""
                s = st[b]
                S = CHS[ch]
                ms = slice((8 + j) * 128, (9 + j) * 128)
                psx = pch.tile([128, 512], f32, tag="chk",
                               name=f"npx{b}_{j}_{ch}")
                for jp in range(2):
                    kp = slice(2 * jp, 2 * jp + 2)
                    nc.tensor.matmul(psx[:], xw8[:, kp, ms],
                                     s["xs8"][:, kp, S],
                                     start=(jp == 0), stop=False,
                                     perf_mode=DR)
                psh = pch.tile([128, 512], f32, tag="chk",
                               name=f"nph{b}_{j}_{ch}")
                for jp in range(2):
                    kp = slice(2 * jp, 2 * jp + 2)
                    nc.tensor.matmul(psh[:], hw8[:, kp, ms],
                                     s["hm8"][:, kp, S],
                                     start=(jp == 0), stop=(jp == 1),
                                     perf_mode=DR)
                t = sb.tile([128, 512], bf16, tag="nt", bufs=4,
                            name=f"nt{b}_{j}_{ch}")
                # t = (psh + S*bhn) * r ; psx += I@t (PE) ;
                # cg = tanh(psx/S + xbn)
                nc.vector.scalar_tensor_tensor(
                    t[:], psh[:], bhnt[:, j:j + 1], s["rg"][j][:, S],
                    OP.add, OP.mult)
                s[f"nt{j}_{ch}"] = (psx, t)

            def emit_nB(b, j, ch):
                s = st[b]
                psx, t = s.pop(f"nt{j}_{ch}")
                nc.tensor.matmul(psx[:], eye[:, :], t[:],
                                 start=False, stop=True)
                nc.scalar.activation(s["cg"][j][:, CHS[ch]], psx[:], AF.Tanh,
                                     bias=xbnt[:, j:j + 1], scale=1.0 / S_SC)

            def stage_B(b, f1=None, f2=None, y1=None, y2=None):
                s = st[b]
                s["rg"] = []
                s["ug"] = [sb.tile([128, F], bf16, tag=f"ug{j}", bufs=1,
                                   name=f"ug{b}_{j}") for j in range(4)]
                s["cg"] = [sb.tile([128, F], bf16, tag=f"cg{j}", bufs=1,
                                   name=f"cg{b}_{j}") for j in range(4)]
                for m in range(F1_POS):
                    emit_rz_gate(b, m)
                if f1 is not None:
                    f1()
                for m in range(F1_POS, 5):
                    emit_rz_gate(b, m)
                emit_nA(b, 0, 0)
                emit_nA(b, 0, 1)
                emit_rz_gate(b, 5)
                emit_nB(b, 0, 0)
                emit_nB(b, 0, 1)
                if f2 is not None and F2_POS == 0:
                    f2()
                emit_nA(b, 1, 0)
                emit_nA(b, 1, 1)
                emit_rz_gate(b, 6)
                emit_nB(b, 1, 0)
                emit_nB(b, 1, 1)
                if f2 is not None and F2_POS == 1:
                    f2()
                emit_nA(b, 2, 0)
                emit_nA(b, 2, 1)
                emit_rz_gate(b, 7)
                emit_nB(b, 2, 0)
                emit_nB(b, 2, 1)
                if f2 is not None and F2_POS == 2:
                    f2()
                emit_nA(b, 3, 0)
                emit_nA(b, 3, 1)
                emit_nB(b, 3, 0)
                emit_nB(b, 3, 1)
                # h_new + x_t  (sub/mul/add on GpSimd, last add on DVE)
                s["y"] = []
                for ct in range(TC):
                    y = sb.tile([128, F], bf16, tag=f"yt{ct}", name=f"yt{b}_{ct}")
                    nc.gpsimd.tensor_sub(y[:], s["ht"][ct][:], s["cg"][ct][:])
                    nc.gpsimd.tensor_mul(y[:], y[:], s["ug"][ct][:])
                    nc.vector.tensor_add(y[:], y[:], s["cg"][ct][:])
                    nc.vector.tensor_add(y[:], y[:], s["xt"][ct][:])
                    s["y"].append(y)

            def stage_Y2(b):
                s = st[b]
                invyb = norm_recip(s["sry"], "y", b)
                for ct in range(TC):
                    if use_won:
                        src = sb.tile([128, F], bf16, tag="yw", bufs=2,
                                      name=f"yw{b}_{ct}")
                        nc.vector.tensor_scalar_mul(src[:], s["y"][ct][:],
                                                    wont[:, ct:ct + 1])
                    else:
                        src = s["y"][ct]
                    o = sb.tile([128, F], bf16, tag="ot", bufs=3,
                                name=f"ot{b}_{ct}")
                    nc.vector.tensor_mul(o[:], src[:], invyb[:])
                    nc.sync.dma_start(
                        outd[b, ct * 128:(ct + 1) * 128, :], o[:])
                st[b] = {}

            # Emission order: the B-stage leads; the F-chain for the next
            # batch is woven into it (F1 after rz2 so its ssq MMs + sqrt
            # run mid-B; F2 after the second n-round) so the V queue
            # reaches F2's ops before the iteration tail.
            stage_L(0)
            for i in range(BPC + 2):
                if i + 1 < BPC:
                    stage_L(i + 1)
                f1 = (lambda i=i: stage_F1(i)) if i < BPC else None
                f2 = (lambda i=i: stage_F2(i)) if i < BPC else None
                if 1 <= i <= BPC:
                    stage_B(i - 1, f1=f1, f2=f2)
                else:
                    for fn in (f1, f2):
                        if fn is not None:
                            fn()
                if i >= 2:
                    stage_Y1(i - 2)
                    stage_Y2(i - 2)

    nc.compile()
    return nc


def _get_program(use_won):
    key = ("nc", use_won)
    if key not in _CACHE:
        _CACHE[key] = _build_program(use_won)
    return _CACHE[key]


def kernel(x_t, h_prev, in_norm_w, hid_norm_w, out_norm_w,
           xW, xb, hmixW, hmixb, hW, hb):
    import ml_dtypes
    from concourse.bass_utils import run_bass_kernel_spmd

    use_won = not np.allclose(np.asarray(out_norm_w, np.float32), 1.0)
    nc = _get_program(use_won)

    f = np.float32
    b16 = ml_dtypes.bfloat16
    f8 = ml_dtypes.float8_e4m3
    x = np.ascontiguousarray(np.asarray(x_t, f).reshape(B, C, F).astype(b16))
    h = np.ascontiguousarray(np.asarray(h_prev, f).reshape(B, C, F).astype(b16))
    xW = np.asarray(xW, f)
    hW = np.asarray(hW, f)

    def quant_w(wT):
        # [C, 3C] scaled, fp8, laid out [128, TC, 3C] with p fastest
        q = np.clip(wT * W_SC, -224.0, 224.0).astype(f8)
        return np.ascontiguousarray(
            q.reshape(TC, 128, 3 * C).transpose(1, 0, 2))

    xWT8 = quant_w((xW * np.asarray(in_norm_w, f)[None, :]).T)
    hWT8 = quant_w(hW.T)
    w3 = np.ascontiguousarray(
        (np.asarray(hmixW, f)[:, 0, 0, :]
         * np.asarray(hid_norm_w, f)[:, None]) * T_SC)
    bh = hW @ np.asarray(hmixb, f) + np.asarray(hb, f)
    gb = np.ascontiguousarray((np.asarray(xb, f) + bh).reshape(3 * C, 1))
    bhn = np.ascontiguousarray(S_SC * bh[2 * C:].reshape(C, 1))
    xbn = np.ascontiguousarray(np.asarray(xb, f)[2 * C:].reshape(C, 1))
    won = np.ascontiguousarray(np.asarray(out_norm_w, f).reshape(C, 1))

    shared = {"xW8": xWT8, "hW8": hWT8, "w3": w3, "gb": gb, "bhn": bhn,
              "xbn": xbn, "won": won,
              "ones_in": np.ones((128, 2, 128), dtype=f8),
              "eye_in": np.eye(128, dtype=b16)}
    in_maps = []
    for c in range(N_CORES):
        m = dict(shared)
        m["x"] = x[c * BPC:(c + 1) * BPC]
        m["h"] = h[c * BPC:(c + 1) * BPC]
        in_maps.append(m)

    res = run_bass_kernel_spmd(nc, in_maps, core_ids=list(range(N_CORES)),
                               **_CACHE.get("run_kwargs", {}))
    _CACHE["last_results"] = res
    out = np.concatenate([res.results[c]["out"] for c in range(N_CORES)],
                         axis=0)
    return out.reshape(B, C, 1, F).astype(np.float32)
